# revision 3
# baseline (speedup 1.0000x reference)
"""Fully-connected GNN message-passing kernel for Trainium2 (8 NeuronCores).

Strategy
--------
The reference graph is fully connected (each graph: all ordered pairs i != j).
This lets us replace gather/segment_sum with dense per-graph math:

  edge-MLP layer 1:  concat([x[i], x[j]]) @ we1 == a_i + b_j
      with a = x @ we1[:H], b = x @ we1[H:]          (tiny matmuls)
  messages for ALL i,j pairs (incl. diagonal) are computed densely;
  agg_i = sum_j silu(silu(a_i+b_j+be1) @ we2 + be2) - diagonal_term_i

Sharding: data-parallel over graphs, 2 graphs per core, weights replicated.
All feature-major ("transposed") layouts on-chip: features on partitions,
nodes/edges along the free dimension.

The scalar engine (ACT) is the roofline: both per-edge SiLU stages run on it
at 1 elem/lane/cycle.  This version maximizes ACTIVATE instruction size to
amortize the ~400-cycle per-instruction overhead:
  PSUM = Z tile [128,2048] f32 (4 banks, z1 slabs, silu1 at FD=2048)
       + M tile [128,1536] f32 (3 banks, edge-MLP2 out, silu2 at FD=1536)
       + SP tile [128,256]  (1 bank, all small matmuls: a/b proj, node MLP)
Biases are folded out of the vector engine: be1/be2 ride the ACTIVATE bias
operand; bn1 is added via a ones-row matmul into the accumulation group.
m1 lives in a persistent per-graph SBUF buffer so silu1 slabs (2048 cols)
and silu2 chunks (1536 cols) can tile the edge space independently.
"""

import numpy as np

# problem shapes (hardcoded per contract)
BS, N, IN_NF, H, EH, OUT_NF, L = 16, 128, 64, 256, 128, 64, 4
NCORES = 8
GPC = BS // NCORES            # graphs per core
NODES = GPC * N               # nodes per core
HC = H // 128                 # H partition chunks
E_G = N * N                   # dense edge count per graph (incl. diagonal)

# edge-pipeline tiling
S1 = 2048                     # silu1 slab columns (16 receivers)
NS1 = E_G // S1               # silu1 slabs per graph (8)
S2 = 1536                     # silu2 chunk columns (12 receivers)
MMQ = 512                     # matmul moving-dim slice (fp32-out bank limit)

MM_DT = "bf16"

_CACHE = {}


def _silu_np(x):
    return x / (1.0 + np.exp(-x))


def _canonical_edges():
    r = np.repeat(np.arange(N), N)
    c = np.tile(np.arange(N), N)
    m = r != c
    r, c = r[m], c[m]
    off = (np.arange(BS) * N)[:, None]
    rows = (r[None, :] + off).reshape(-1)
    cols = (c[None, :] + off).reshape(-1)
    return rows, cols


def _edges_match(rows, cols):
    """True if (rows, cols) describe the canonical fully-connected batch."""
    er, ec = _canonical_edges()
    rows = np.asarray(rows).astype(np.int64).ravel()
    cols = np.asarray(cols).astype(np.int64).ravel()
    if rows.shape != er.shape or cols.shape != ec.shape:
        return False
    if np.array_equal(rows, er) and np.array_equal(cols, ec):
        return True
    # permuted edge list: compare sorted edge keys
    k1 = np.sort(rows * (BS * N) + cols)
    k2 = np.sort(er * (BS * N) + ec)
    return np.array_equal(k1, k2)


def _numpy_reference(h, rows, cols, w_in, b_in, w_out, b_out,
                     we1, be1, we2, be2, wn1, bn1, wn2, bn2):
    """Exact fallback (only used if inputs are not the canonical FC batch)."""
    f = np.float32
    x = h.reshape(BS * N, -1).astype(f) @ w_in.astype(f) + b_in.astype(f)
    rows = np.asarray(rows).astype(np.int64)
    cols = np.asarray(cols).astype(np.int64)
    for l in range(L):
        m = np.concatenate([x[rows], x[cols]], axis=-1)
        m = _silu_np(m @ we1[l].astype(f) + be1[l].astype(f))
        m = _silu_np(m @ we2[l].astype(f) + be2[l].astype(f))
        agg = np.zeros((BS * N, m.shape[-1]), f)
        np.add.at(agg, rows, m)
        u = np.concatenate([x, agg], axis=-1)
        u = _silu_np(u @ wn1[l].astype(f) + bn1[l].astype(f))
        u = u @ wn2[l].astype(f) + bn2[l].astype(f)
        x = x + u
    return x @ w_out.astype(f) + b_out.astype(f)


def _split_excess_waits(nc, mybir, cap=1):
    """The walrus build in this environment accepts only one sync-wait per
    instruction; move extra waits onto preceding same-engine NOPs."""
    n_split = 0
    for fn in nc.m.functions:
        for blk in fn.blocks:
            il = blk.instructions
            new = []
            changed = False
            for ins in il:
                si = ins.sync_info
                if si is not None and si.on_wait and len(si.on_wait) > cap:
                    waits = list(si.on_wait)
                    keep, extra = waits[-cap:], waits[:-cap]
                    for w in extra:
                        nop = mybir.InstNoOp(name=f"I-wsplit-{nc.next_id()}",
                                             ins=[], outs=[])
                        nop.engine = ins.engine
                        nop.sync_info = mybir.SyncInfo(on_wait=[w], on_update=[])
                        new.append(nop)
                        n_split += 1
                    ins.sync_info = mybir.SyncInfo(on_wait=keep,
                                                   on_update=list(si.on_update))
                    changed = True
                new.append(ins)
            if changed:
                il[:] = new
    return n_split


def _chunks():
    """silu2 chunk list per graph: (receiver offset, receiver count)."""
    out = []
    r = 0
    while r < N:
        nr = min(S2 // N, N - r)
        out.append((r, nr))
        r += nr
    return out


def _build_nc(split_waits=True):
    import concourse.bass as bass
    import concourse.tile as tile
    import concourse.mybir as mybir
    from contextlib import ExitStack

    f32 = mybir.dt.float32
    bf16 = mybir.dt.bfloat16
    mdt = f32 if MM_DT == "f32" else bf16
    AF = mybir.ActivationFunctionType
    ALU = mybir.AluOpType

    nc = bass.Bass()

    # ---- DRAM parameters (per core) ----
    h_d = nc.declare_dram_parameter("h_c", [NODES, IN_NF], f32, isOutput=False)
    w_in_d = nc.declare_dram_parameter("w_in", [IN_NF, H], f32, isOutput=False)
    b_in_d = nc.declare_dram_parameter("b_in", [H], f32, isOutput=False)
    w_out_d = nc.declare_dram_parameter("w_out", [H, OUT_NF], f32, isOutput=False)
    b_out_d = nc.declare_dram_parameter("b_out", [OUT_NF], f32, isOutput=False)
    we1_d = nc.declare_dram_parameter("we1", [L, 2 * H, EH], f32, isOutput=False)
    be1_d = nc.declare_dram_parameter("be1", [L, EH], f32, isOutput=False)
    we2_d = nc.declare_dram_parameter("we2", [L, EH, EH], mdt, isOutput=False)
    be2_d = nc.declare_dram_parameter("be2", [L, EH], f32, isOutput=False)
    wn1_d = nc.declare_dram_parameter("wn1", [L, H + EH, H], f32, isOutput=False)
    bn1_d = nc.declare_dram_parameter("bn1", [L, H], f32, isOutput=False)
    wn2_d = nc.declare_dram_parameter("wn2", [L, H, H], f32, isOutput=False)
    bn2_d = nc.declare_dram_parameter("bn2", [L, H], f32, isOutput=False)
    ident_d = nc.declare_dram_parameter("ident", [128, 128], mdt, isOutput=False)
    identf_d = nc.declare_dram_parameter("identf", [128, 128], f32, isOutput=False)
    ones_d = nc.declare_dram_parameter("ones_r", [1, 128], f32, isOutput=False)
    out_d = nc.declare_dram_parameter("out_c", [NODES, OUT_NF], f32, isOutput=True)

    CH = _chunks()                  # silu2 chunks per graph
    NCH = len(CH)

    with tile.TileContext(nc) as tc, ExitStack() as ctx:
        consts = ctx.enter_context(tc.tile_pool(name="consts", bufs=1))
        work = ctx.enter_context(tc.tile_pool(name="work", bufs=2))
        mpool = ctx.enter_context(tc.tile_pool(name="mp", bufs=2))
        xpool = ctx.enter_context(tc.tile_pool(name="xp", bufs=2))
        psum = ctx.enter_context(tc.tile_pool(name="ps", bufs=1, space="PSUM"))

        dma = nc.sync.dma_start

        # warm the ACT Silu table immediately (zero-dependency dummy op)
        warm = work.tile([1, 2], f32, tag="warm", name="warm")
        nc.vector.memset(warm[0:1, 0:1], 0.0)
        nc.scalar.activation(warm[0:1, 1:2], warm[0:1, 0:1], AF.Silu)

        # ---- input loads (h first: it heads the critical path) ----
        hns = []
        for nb in range(NODES // 128):
            hn = work.tile([128, IN_NF], f32, tag="hn", name=f"hn_{nb}")
            dma(out=hn[:], in_=h_d[nb * 128:(nb + 1) * 128, :])
            hns.append(hn)

        # ---- constant loads ----
        identf_sb = consts.tile([128, 128], f32, tag="identf", name="identf_sb")
        dma(out=identf_sb[:], in_=identf_d[:])
        w_in_sb = consts.tile([IN_NF, H], f32, tag="w_in", name="w_in_sb")
        dma(out=w_in_sb[:], in_=w_in_d[:])
        b_in_sb = consts.tile([128, HC], f32, tag="b_in", name="b_in_sb")
        dma(out=b_in_sb[:], in_=b_in_d.rearrange("(m p) -> p m", p=128))

        if MM_DT == "f32":
            ident_sb = identf_sb
        else:
            ident_sb = consts.tile([128, 128], mdt, tag="ident", name="ident_sb")
            dma(out=ident_sb[:], in_=ident_d[:])

        # ---- input embedding: x_T[m] = (h @ w_in + b_in)^T ----
        # psum traffic goes through the big Z/M tags (free until slab 0).
        hT = work.tile([IN_NF, NODES], f32, tag="hT", name="hT")
        for nb in range(NODES // 128):
            hTp = psum.tile([IN_NF, 128], f32, tag=("Z", "M")[nb], name=f"hTp_{nb}")
            nc.tensor.transpose(hTp[:], hns[nb][:], identf_sb[:])
            nc.vector.tensor_copy(hT[:, nb * 128:(nb + 1) * 128], hTp[:])

        x_T = [xpool.tile([128, NODES], f32, tag=f"x{m}", name=f"x0_{m}")
               for m in range(HC)]
        for g in range(GPC):
            gb = slice(g * N, (g + 1) * N)
            for m in range(HC):
                xp_ = psum.tile([128, N], f32, tag=("Z", "M")[m],
                                name=f"xemb_{g}_{m}")
                nc.tensor.matmul(xp_[:], lhsT=w_in_sb[:, m * 128:(m + 1) * 128],
                                 rhs=hT[:, gb], start=True, stop=True)
                nc.vector.tensor_scalar_add(x_T[m][:, gb], xp_[:], b_in_sb[:, m:m + 1])

        # ---- weight loads ----
        we1_sb, we2_sb, wn1_sb, wn2_sb = [], [], [], []
        be1_sb = consts.tile([EH, L], f32, tag="be1", name="be1_sb")
        dma(out=be1_sb[:], in_=be1_d.rearrange("l p -> p l"))
        be2_sb = consts.tile([EH, L], f32, tag="be2", name="be2_sb")
        dma(out=be2_sb[:], in_=be2_d.rearrange("l p -> p l"))
        # bn1 as a flat row for the bias-fold matmul (lhsT=[1,128] slice)
        bn1_row = consts.tile([1, L * H], f32, tag="bn1r", name="bn1_row")
        dma(out=bn1_row[:], in_=bn1_d.rearrange("l h -> (l h)").unsqueeze(0))
        bn2_sb = consts.tile([128, L * HC], f32, tag="bn2", name="bn2_sb")
        dma(out=bn2_sb[:], in_=bn2_d.rearrange("l (m p) -> p (l m)", p=128))
        for l in range(L):
            t1 = []
            for j in range(4):
                t = consts.tile([128, EH], f32, tag=f"we1_{l}_{j}", name=f"we1_{l}_{j}")
                dma(out=t[:], in_=we1_d[l, j * 128:(j + 1) * 128, :])
                t1.append(t)
            we1_sb.append(t1)
            t = consts.tile([EH, EH], mdt, tag=f"we2_{l}", name=f"we2_{l}")
            dma(out=t[:], in_=we2_d[l])
            we2_sb.append(t)
            tn = []
            for k in range(3):
                t = consts.tile([128, H], f32, tag=f"wn1_{l}_{k}", name=f"wn1_{l}_{k}")
                dma(out=t[:], in_=wn1_d[l, k * 128:(k + 1) * 128, :])
                tn.append(t)
            wn1_sb.append(tn)
            tn = []
            for k in range(2):
                t = consts.tile([128, H], f32, tag=f"wn2_{l}_{k}", name=f"wn2_{l}_{k}")
                dma(out=t[:], in_=wn2_d[l, k * 128:(k + 1) * 128, :])
                tn.append(t)
            wn2_sb.append(tn)
        w_out_sb = []
        for k in range(HC):
            t = consts.tile([128, OUT_NF], f32, tag=f"w_out_{k}", name=f"w_out_{k}")
            dma(out=t[:], in_=w_out_d[k * 128:(k + 1) * 128, :])
            w_out_sb.append(t)
        b_out_sb = consts.tile([1, OUT_NF], f32, tag="b_out", name="b_out_sb")
        dma(out=b_out_sb[:], in_=b_out_d[:].unsqueeze(0))
        ones_sb = consts.tile([1, 128], f32, tag="ones", name="ones_sb")
        dma(out=ones_sb[:], in_=ones_d[:])

        # ---- per-(layer, graph) state ----
        abT = {}                    # (l, g) -> a-projection [EH, N] bf16
        b512 = {}                   # (l, g) -> b replicated x4 [EH, 512] bf16
        m1buf = {}                  # g -> persistent m1 [EH, E_G] bf16
        aggT = {}                   # l -> [EH, NODES]
        diagT = {}                  # l -> [EH, NODES]
        x_cur = {0: x_T}            # l -> x_T tiles
        RPQ = MMQ // N              # receivers per z1-build matmul block (4)

        def emit_ab_proj(l, g):
            gb = slice(g * N, (g + 1) * N)
            xT = x_cur[l]
            apbp = psum.tile([EH, 256], f32, tag="SP", name=f"apbp_{l}_{g}")
            for m in range(HC):
                nc.tensor.matmul(apbp[:, 0:128], lhsT=we1_sb[l][m][:],
                                 rhs=xT[m][:, gb],
                                 start=(m == 0), stop=(m == HC - 1))
            for m in range(HC):
                nc.tensor.matmul(apbp[:, 128:256], lhsT=we1_sb[l][HC + m][:],
                                 rhs=xT[m][:, gb],
                                 start=(m == 0), stop=(m == HC - 1))
            a = work.tile([EH, N], mdt, tag=f"abT{g}", name=f"abT_{l}_{g}")
            nc.vector.tensor_copy(a[:], apbp[:, 0:128])
            abT[(l, g)] = a
            b = work.tile([EH, MMQ], mdt, tag=f"b512_{g}", name=f"b512_{l}_{g}")
            nc.vector.tensor_copy(b[:, 0:128], apbp[:, 128:256])
            nc.vector.tensor_copy(b[:, 128:256], b[:, 0:128])
            nc.vector.tensor_copy(b[:, 256:512], b[:, 0:256])
            b512[(l, g)] = b

        def emit_slab(l, g, s):
            # build z1 slab in Z (PE), silu1 -> m1buf (ACT, FD=S1)
            r0 = s * (S1 // N)
            Z = psum.tile([EH, S1], f32, tag="Z", name=f"z_{l}_{g}_{s}")
            for q in range(S1 // MMQ):
                base = r0 + RPQ * q
                rhs_a = abT[(l, g)][:, base: base + RPQ] \
                    .unsqueeze(2).broadcast_to([EH, RPQ, N])
                nc.tensor.matmul(Z[:, q * MMQ:(q + 1) * MMQ],
                                 lhsT=ident_sb[:], rhs=rhs_a,
                                 start=True, stop=False)
                nc.tensor.matmul(Z[:, q * MMQ:(q + 1) * MMQ],
                                 lhsT=ident_sb[:], rhs=b512[(l, g)][:],
                                 start=False, stop=True)
            nc.scalar.activation(m1buf[g][:, s * S1:(s + 1) * S1], Z[:],
                                 AF.Silu, bias=be1_sb[:, l:l + 1])

        def emit_chunk(l, g, f):
            # edge-MLP2 (PE) -> M, silu2 (ACT, FD=w), reduce+diag (DVE)
            r0, nr = CH[f]
            w = nr * N
            c0 = r0 * N
            M = psum.tile([EH, w], f32, tag="M", name=f"m_{l}_{g}_{f}")
            for q in range(w // MMQ):
                nc.tensor.matmul(M[:, q * MMQ:(q + 1) * MMQ],
                                 lhsT=we2_sb[l][:],
                                 rhs=m1buf[g][:, c0 + q * MMQ: c0 + (q + 1) * MMQ],
                                 start=True, stop=True)
            m2 = mpool.tile([EH, w], f32, tag="m2", bufs=3, name=f"m2_{l}_{g}_{f}")
            nc.scalar.activation(m2[:], M[:], AF.Silu, bias=be2_sb[:, l:l + 1])
            red_in = m2[:].rearrange("p (i j) -> p i j", j=N)
            nc.vector.tensor_reduce(
                aggT[l][:, g * N + r0: g * N + r0 + nr],
                red_in, axis=mybir.AxisListType.X, op=ALU.add)
            # diagonal (j == i) messages for correction: receiver r0+t's own
            # column within this chunk is t*N + (r0+t)
            diag_ap = bass.AP(
                tensor=m2.tensor, offset=m2.offset + r0,
                ap=[m2.ap[0], [N + 1, nr]])
            nc.vector.tensor_copy(
                diagT[l][:, g * N + r0: g * N + r0 + nr], diag_ap)

        def emit_node_mlp(l, g):
            # subtract diagonal, node MLP (bn1 folded via ones-matmul),
            # residual into x_cur[l+1]
            gb = slice(g * N, (g + 1) * N)
            xT = x_cur[l]
            xN = x_cur[l + 1]
            nc.vector.tensor_sub(aggT[l][:, gb], aggT[l][:, gb], diagT[l][:, gb])
            up = psum.tile([128, 256], f32, tag="SP", name=f"up_{l}_{g}")
            for m in range(HC):
                mc = slice(m * 128, (m + 1) * 128)
                nc.tensor.matmul(up[:, mc], lhsT=wn1_sb[l][0][:, m * 128:(m + 1) * 128],
                                 rhs=xT[0][:, gb], start=True, stop=False)
                nc.tensor.matmul(up[:, mc], lhsT=wn1_sb[l][1][:, m * 128:(m + 1) * 128],
                                 rhs=xT[1][:, gb], start=False, stop=False)
                nc.tensor.matmul(up[:, mc], lhsT=wn1_sb[l][2][:, m * 128:(m + 1) * 128],
                                 rhs=aggT[l][:, gb], start=False, stop=False)
                nc.tensor.matmul(up[:, mc],
                                 lhsT=bn1_row[0:1, l * H + m * 128: l * H + (m + 1) * 128],
                                 rhs=ones_sb[0:1, 0:128], start=False, stop=True)
            u1c = work.tile([128, 256], f32, tag="u1c", name=f"u1c_{l}_{g}")
            nc.scalar.activation(u1c[:], up[:], AF.Silu)
            u2 = psum.tile([128, 256], f32, tag="SP", name=f"u2_{l}_{g}")
            for m in range(HC):
                mc = slice(m * 128, (m + 1) * 128)
                nc.tensor.matmul(u2[:, mc], lhsT=wn2_sb[l][0][:, m * 128:(m + 1) * 128],
                                 rhs=u1c[:, 0:128], start=True, stop=False)
                nc.tensor.matmul(u2[:, mc], lhsT=wn2_sb[l][1][:, m * 128:(m + 1) * 128],
                                 rhs=u1c[:, 128:256], start=False, stop=True)
            for m in range(HC):
                mc = slice(m * 128, (m + 1) * 128)
                nc.vector.scalar_tensor_tensor(
                    xN[m][:, gb], u2[:, mc],
                    bn2_sb[:, l * HC + m: l * HC + m + 1], xT[m][:, gb],
                    op0=ALU.add, op1=ALU.add)

        # ---- flat software-pipelined emission across (layer, graph) ----
        for g in range(GPC):
            m1buf[g] = work.tile([EH, E_G], mdt, tag=f"m1_{g}", bufs=1,
                                 name=f"m1buf_{g}")
        for l in range(L):
            aggT[l] = work.tile([EH, NODES], f32, tag="agg", name=f"agg_{l}")
            diagT[l] = work.tile([EH, NODES], f32, tag="diag", name=f"diag_{l}")
            x_cur[l + 1] = [xpool.tile([128, NODES], f32, tag=f"x{m}",
                                       name=f"x{l + 1}_{m}")
                            for m in range(HC)]

        slabs = [(l, g, s) for l in range(L) for g in range(GPC)
                 for s in range(NS1)]
        chunks = [(l, g, f) for l in range(L) for g in range(GPC)
                  for f in range(NCH)]
        covered = {}
        si = ci = 0
        last_was_chunk = False
        while ci < len(chunks):
            lc, gc, fc = chunks[ci]
            emit_s = False
            if si < len(slabs):
                ls, gs, ss = slabs[si]
                need = (CH[fc][0] + CH[fc][1]) * N
                if (ls, gs) == (lc, gc):
                    emit_s = covered.get((ls, gs), 0) < min(need + S1, E_G + 1)
                else:
                    emit_s = last_was_chunk
            if emit_s:
                if ss == 0:
                    emit_ab_proj(ls, gs)
                emit_slab(ls, gs, ss)
                covered[(ls, gs)] = (ss + 1) * S1
                si += 1
                last_was_chunk = False
            else:
                emit_chunk(lc, gc, fc)
                ci += 1
                last_was_chunk = True
                if fc == NCH - 1:
                    emit_node_mlp(lc, gc)
        while si < len(slabs):   # safety (should not trigger)
            ls, gs, ss = slabs[si]
            if ss == 0:
                emit_ab_proj(ls, gs)
            emit_slab(ls, gs, ss)
            si += 1

        # ---- output embedding: out = x @ w_out + b_out (natural layout) ----
        xF = x_cur[L]
        for nb in range(NODES // 128):
            op_ = psum.tile([128, OUT_NF], f32, tag=("Z", "M")[nb],
                            name=f"outp_{nb}")
            nc.tensor.matmul(op_[:], lhsT=xF[0][:, nb * 128:(nb + 1) * 128],
                             rhs=w_out_sb[0][:], start=True, stop=False)
            nc.tensor.matmul(op_[:], lhsT=xF[1][:, nb * 128:(nb + 1) * 128],
                             rhs=w_out_sb[1][:], start=False, stop=False)
            nc.tensor.matmul(op_[:], lhsT=ones_sb[0:1, 0:128], rhs=b_out_sb[0:1, :],
                             start=False, stop=True)
            ob = work.tile([128, OUT_NF], f32, tag="ob", name=f"ob_{nb}")
            nc.vector.tensor_copy(ob[:], op_[:])
            dma(out=out_d[nb * 128:(nb + 1) * 128, :], in_=ob[:])

    if split_waits:
        _split_excess_waits(nc, mybir)
    return nc


def _get_nc():
    if "nc" not in _CACHE:
        _CACHE["nc"] = _build_nc()
    return _CACHE["nc"]


def _to_mdt(a):
    if MM_DT == "bf16":
        import ml_dtypes
        return np.asarray(a, dtype=np.float32).astype(ml_dtypes.bfloat16)
    return np.asarray(a, dtype=np.float32)


def _run_on_hw(inputs, **spmd_kwargs):
    """Shard, run on the 8 NeuronCores, gather. Returns (out, BassKernelResults)."""
    from concourse.bass_utils import run_bass_kernel_spmd

    f = np.float32
    h = np.ascontiguousarray(np.asarray(inputs["h"], dtype=f))
    ws = {k: np.ascontiguousarray(np.asarray(inputs[k], dtype=f))
          for k in ("w_in", "b_in", "w_out", "b_out", "we1", "be1", "we2",
                    "be2", "wn1", "bn1", "wn2", "bn2")}
    nc = _get_nc()
    base = {
        "w_in": ws["w_in"], "b_in": ws["b_in"],
        "w_out": ws["w_out"], "b_out": ws["b_out"],
        "we1": ws["we1"], "be1": ws["be1"],
        "we2": _to_mdt(ws["we2"]), "be2": ws["be2"],
        "wn1": ws["wn1"], "bn1": ws["bn1"],
        "wn2": ws["wn2"], "bn2": ws["bn2"],
        "ident": _to_mdt(np.eye(128, dtype=f)),
        "identf": np.eye(128, dtype=f),
        "ones_r": np.ones((1, 128), dtype=f),
    }
    in_maps = []
    for c in range(NCORES):
        m = dict(base)
        m["h_c"] = np.ascontiguousarray(
            h[c * GPC:(c + 1) * GPC].reshape(NODES, IN_NF))
        in_maps.append(m)

    res = run_bass_kernel_spmd(nc, in_maps, list(range(NCORES)), **spmd_kwargs)
    out = np.concatenate([np.asarray(res.results[i]["out_c"], dtype=f)
                          for i in range(NCORES)], axis=0)
    return out, res


def kernel(**inputs):
    h = np.asarray(inputs["h"])
    rows, cols = inputs["rows"], inputs["cols"]
    if h.shape != (BS, N, IN_NF) or not _edges_match(rows, cols):
        ws = {k: np.asarray(inputs[k], dtype=np.float32)
              for k in ("w_in", "b_in", "w_out", "b_out", "we1", "be1", "we2",
                        "be2", "wn1", "bn1", "wn2", "bn2")}
        return _numpy_reference(np.asarray(h, np.float32), rows, cols, **ws)
    out, _ = _run_on_hw(inputs)
    return out


# revision 4
# speedup vs baseline: 1.0564x; 1.0564x over previous
"""Fully-connected GNN message-passing kernel for Trainium2 (8 NeuronCores).

Strategy
--------
The reference graph is fully connected (each graph: all ordered pairs i != j).
This lets us replace gather/segment_sum with dense per-graph math:

  edge-MLP layer 1:  concat([x[i], x[j]]) @ we1 == a_i + b_j
      with a = x @ we1[:H], b = x @ we1[H:]          (tiny matmuls)
  messages for ALL i,j pairs (incl. diagonal) are computed densely;
  agg_i = sum_j silu(silu(a_i+b_j+be1) @ we2 + be2) - diagonal_term_i

Sharding: data-parallel over graphs, 2 graphs per core, weights replicated.
All feature-major ("transposed") layouts on-chip: features on partitions,
nodes/edges along the free dimension.

The scalar engine (ACT) is the roofline: both per-edge SiLU stages run on it
at 1 elem/lane/cycle.  This version maximizes ACTIVATE instruction size to
amortize the ~400-cycle per-instruction overhead:
  PSUM = Z tile [128,2048] f32 (4 banks, z1 slabs, silu1 at FD=2048)
       + M tile [128,1536] f32 (3 banks, edge-MLP2 out, silu2 at FD=1536)
       + SP tile [128,256]  (1 bank, all small matmuls: a/b proj, node MLP)
Biases are folded out of the vector engine: be1/be2 ride the ACTIVATE bias
operand; bn1 is added via a ones-row matmul into the accumulation group.
m1 lives in a persistent per-graph SBUF buffer so silu1 slabs (2048 cols)
and silu2 chunks (1536 cols) can tile the edge space independently.
"""

import numpy as np

# problem shapes (hardcoded per contract)
BS, N, IN_NF, H, EH, OUT_NF, L = 16, 128, 64, 256, 128, 64, 4
NCORES = 8
GPC = BS // NCORES            # graphs per core
NODES = GPC * N               # nodes per core
HC = H // 128                 # H partition chunks
E_G = N * N                   # dense edge count per graph (incl. diagonal)

# edge-pipeline tiling
S1 = 2048                     # silu1 slab columns (16 receivers)
NS1 = E_G // S1               # silu1 slabs per graph (8)
S2 = 1536                     # silu2 chunk columns (12 receivers)
MMQ = 512                     # matmul moving-dim slice (fp32-out bank limit)

MM_DT = "bf16"

_CACHE = {}


def _silu_np(x):
    return x / (1.0 + np.exp(-x))


def _canonical_edges():
    r = np.repeat(np.arange(N), N)
    c = np.tile(np.arange(N), N)
    m = r != c
    r, c = r[m], c[m]
    off = (np.arange(BS) * N)[:, None]
    rows = (r[None, :] + off).reshape(-1)
    cols = (c[None, :] + off).reshape(-1)
    return rows, cols


def _edges_match(rows, cols):
    """True if (rows, cols) describe the canonical fully-connected batch."""
    er, ec = _canonical_edges()
    rows = np.asarray(rows).astype(np.int64).ravel()
    cols = np.asarray(cols).astype(np.int64).ravel()
    if rows.shape != er.shape or cols.shape != ec.shape:
        return False
    if np.array_equal(rows, er) and np.array_equal(cols, ec):
        return True
    # permuted edge list: compare sorted edge keys
    k1 = np.sort(rows * (BS * N) + cols)
    k2 = np.sort(er * (BS * N) + ec)
    return np.array_equal(k1, k2)


def _numpy_reference(h, rows, cols, w_in, b_in, w_out, b_out,
                     we1, be1, we2, be2, wn1, bn1, wn2, bn2):
    """Exact fallback (only used if inputs are not the canonical FC batch)."""
    f = np.float32
    x = h.reshape(BS * N, -1).astype(f) @ w_in.astype(f) + b_in.astype(f)
    rows = np.asarray(rows).astype(np.int64)
    cols = np.asarray(cols).astype(np.int64)
    for l in range(L):
        m = np.concatenate([x[rows], x[cols]], axis=-1)
        m = _silu_np(m @ we1[l].astype(f) + be1[l].astype(f))
        m = _silu_np(m @ we2[l].astype(f) + be2[l].astype(f))
        agg = np.zeros((BS * N, m.shape[-1]), f)
        np.add.at(agg, rows, m)
        u = np.concatenate([x, agg], axis=-1)
        u = _silu_np(u @ wn1[l].astype(f) + bn1[l].astype(f))
        u = u @ wn2[l].astype(f) + bn2[l].astype(f)
        x = x + u
    return x @ w_out.astype(f) + b_out.astype(f)


def _split_excess_waits(nc, mybir, cap=1):
    """The walrus build in this environment accepts only one sync-wait per
    instruction; move extra waits onto preceding same-engine NOPs."""
    n_split = 0
    for fn in nc.m.functions:
        for blk in fn.blocks:
            il = blk.instructions
            new = []
            changed = False
            for ins in il:
                si = ins.sync_info
                if si is not None and si.on_wait and len(si.on_wait) > cap:
                    waits = list(si.on_wait)
                    keep, extra = waits[-cap:], waits[:-cap]
                    for w in extra:
                        nop = mybir.InstNoOp(name=f"I-wsplit-{nc.next_id()}",
                                             ins=[], outs=[])
                        nop.engine = ins.engine
                        nop.sync_info = mybir.SyncInfo(on_wait=[w], on_update=[])
                        new.append(nop)
                        n_split += 1
                    ins.sync_info = mybir.SyncInfo(on_wait=keep,
                                                   on_update=list(si.on_update))
                    changed = True
                new.append(ins)
            if changed:
                il[:] = new
    return n_split


def _chunks():
    """silu2 chunk list per graph: (receiver offset, receiver count)."""
    out = []
    r = 0
    while r < N:
        nr = min(S2 // N, N - r)
        out.append((r, nr))
        r += nr
    return out


def _build_nc(split_waits=True):
    import concourse.bass as bass
    import concourse.tile as tile
    import concourse.mybir as mybir
    from contextlib import ExitStack

    f32 = mybir.dt.float32
    bf16 = mybir.dt.bfloat16
    mdt = f32 if MM_DT == "f32" else bf16
    AF = mybir.ActivationFunctionType
    ALU = mybir.AluOpType

    nc = bass.Bass()

    # ---- DRAM parameters (per core) ----
    h_d = nc.declare_dram_parameter("h_c", [NODES, IN_NF], f32, isOutput=False)
    w_in_d = nc.declare_dram_parameter("w_in", [IN_NF, H], f32, isOutput=False)
    b_in_d = nc.declare_dram_parameter("b_in", [H], f32, isOutput=False)
    w_out_d = nc.declare_dram_parameter("w_out", [H, OUT_NF], f32, isOutput=False)
    b_out_d = nc.declare_dram_parameter("b_out", [OUT_NF], f32, isOutput=False)
    we1_d = nc.declare_dram_parameter("we1", [L, 2 * H, EH], f32, isOutput=False)
    be1_d = nc.declare_dram_parameter("be1", [L, EH], f32, isOutput=False)
    we2_d = nc.declare_dram_parameter("we2", [L, EH, EH], mdt, isOutput=False)
    be2_d = nc.declare_dram_parameter("be2", [L, EH], f32, isOutput=False)
    wn1_d = nc.declare_dram_parameter("wn1", [L, H + EH, H], f32, isOutput=False)
    bn1_d = nc.declare_dram_parameter("bn1", [L, H], f32, isOutput=False)
    wn2_d = nc.declare_dram_parameter("wn2", [L, H, H], f32, isOutput=False)
    bn2_d = nc.declare_dram_parameter("bn2", [L, H], f32, isOutput=False)
    ident_d = nc.declare_dram_parameter("ident", [128, 128], mdt, isOutput=False)
    identf_d = nc.declare_dram_parameter("identf", [128, 128], f32, isOutput=False)
    ones_d = nc.declare_dram_parameter("ones_r", [1, 128], f32, isOutput=False)
    out_d = nc.declare_dram_parameter("out_c", [NODES, OUT_NF], f32, isOutput=True)

    CH = _chunks()                  # silu2 chunks per graph
    NCH = len(CH)

    with tile.TileContext(nc) as tc, ExitStack() as ctx:
        consts = ctx.enter_context(tc.tile_pool(name="consts", bufs=1))
        work = ctx.enter_context(tc.tile_pool(name="work", bufs=2))
        mpool = ctx.enter_context(tc.tile_pool(name="mp", bufs=2))
        xpool = ctx.enter_context(tc.tile_pool(name="xp", bufs=2))
        psum = ctx.enter_context(tc.tile_pool(name="ps", bufs=1, space="PSUM"))

        dma = nc.sync.dma_start

        # warm the ACT Silu table immediately (zero-dependency dummy op)
        warm = work.tile([1, 2], f32, tag="warm", name="warm")
        nc.vector.memset(warm[0:1, 0:1], 0.0)
        nc.scalar.activation(warm[0:1, 1:2], warm[0:1, 0:1], AF.Silu)

        # ---- input loads (h first: it heads the critical path) ----
        hns = []
        for nb in range(NODES // 128):
            hn = work.tile([128, IN_NF], f32, tag="hn", name=f"hn_{nb}")
            dma(out=hn[:], in_=h_d[nb * 128:(nb + 1) * 128, :])
            hns.append(hn)

        # ---- constant loads ----
        identf_sb = consts.tile([128, 128], f32, tag="identf", name="identf_sb")
        dma(out=identf_sb[:], in_=identf_d[:])
        w_in_sb = consts.tile([IN_NF, H], f32, tag="w_in", name="w_in_sb")
        dma(out=w_in_sb[:], in_=w_in_d[:])
        b_in_sb = consts.tile([128, HC], f32, tag="b_in", name="b_in_sb")
        dma(out=b_in_sb[:], in_=b_in_d.rearrange("(m p) -> p m", p=128))

        if MM_DT == "f32":
            ident_sb = identf_sb
        else:
            ident_sb = consts.tile([128, 128], mdt, tag="ident", name="ident_sb")
            dma(out=ident_sb[:], in_=ident_d[:])

        # ---- input embedding: x_T[m] = (h @ w_in + b_in)^T ----
        # psum traffic goes through the big Z/M tags (free until slab 0).
        hT = work.tile([IN_NF, NODES], f32, tag="hT", name="hT")
        for nb in range(NODES // 128):
            hTp = psum.tile([IN_NF, 128], f32, tag=("Z", "M")[nb], name=f"hTp_{nb}")
            nc.tensor.transpose(hTp[:], hns[nb][:], identf_sb[:])
            nc.vector.tensor_copy(hT[:, nb * 128:(nb + 1) * 128], hTp[:])

        x_T = [xpool.tile([128, NODES], f32, tag=f"x{m}", name=f"x0_{m}")
               for m in range(HC)]
        for g in range(GPC):
            gb = slice(g * N, (g + 1) * N)
            for m in range(HC):
                xp_ = psum.tile([128, N], f32, tag=("Z", "M")[m],
                                name=f"xemb_{g}_{m}")
                nc.tensor.matmul(xp_[:], lhsT=w_in_sb[:, m * 128:(m + 1) * 128],
                                 rhs=hT[:, gb], start=True, stop=True)
                nc.vector.tensor_scalar_add(x_T[m][:, gb], xp_[:], b_in_sb[:, m:m + 1])

        # ---- weight loads ----
        we1_sb, we2_sb, wn1_sb, wn2_sb = [], [], [], []
        be1_sb = consts.tile([EH, L], f32, tag="be1", name="be1_sb")
        dma(out=be1_sb[:], in_=be1_d.rearrange("l p -> p l"))
        be2_sb = consts.tile([EH, L], f32, tag="be2", name="be2_sb")
        dma(out=be2_sb[:], in_=be2_d.rearrange("l p -> p l"))
        # bn1 as a flat row for the bias-fold matmul (lhsT=[1,128] slice)
        bn1_row = consts.tile([1, L * H], f32, tag="bn1r", name="bn1_row")
        dma(out=bn1_row[:], in_=bn1_d.rearrange("l h -> (l h)").unsqueeze(0))
        bn2_sb = consts.tile([128, L * HC], f32, tag="bn2", name="bn2_sb")
        dma(out=bn2_sb[:], in_=bn2_d.rearrange("l (m p) -> p (l m)", p=128))
        for l in range(L):
            t1 = []
            for j in range(4):
                t = consts.tile([128, EH], f32, tag=f"we1_{l}_{j}", name=f"we1_{l}_{j}")
                dma(out=t[:], in_=we1_d[l, j * 128:(j + 1) * 128, :])
                t1.append(t)
            we1_sb.append(t1)
            t = consts.tile([EH, EH], mdt, tag=f"we2_{l}", name=f"we2_{l}")
            dma(out=t[:], in_=we2_d[l])
            we2_sb.append(t)
            tn = []
            for k in range(3):
                t = consts.tile([128, H], f32, tag=f"wn1_{l}_{k}", name=f"wn1_{l}_{k}")
                dma(out=t[:], in_=wn1_d[l, k * 128:(k + 1) * 128, :])
                tn.append(t)
            wn1_sb.append(tn)
            tn = []
            for k in range(2):
                t = consts.tile([128, H], f32, tag=f"wn2_{l}_{k}", name=f"wn2_{l}_{k}")
                dma(out=t[:], in_=wn2_d[l, k * 128:(k + 1) * 128, :])
                tn.append(t)
            wn2_sb.append(tn)
        w_out_sb = []
        for k in range(HC):
            t = consts.tile([128, OUT_NF], f32, tag=f"w_out_{k}", name=f"w_out_{k}")
            dma(out=t[:], in_=w_out_d[k * 128:(k + 1) * 128, :])
            w_out_sb.append(t)
        b_out_sb = consts.tile([1, OUT_NF], f32, tag="b_out", name="b_out_sb")
        dma(out=b_out_sb[:], in_=b_out_d[:].unsqueeze(0))
        ones_sb = consts.tile([1, 128], f32, tag="ones", name="ones_sb")
        dma(out=ones_sb[:], in_=ones_d[:])

        # ---- per-(layer, graph) state ----
        abT = {}                    # (l, g) -> a-projection [EH, N] bf16
        b512 = {}                   # (l, g) -> b replicated x4 [EH, 512] bf16
        m1buf = {}                  # g -> persistent m1 [EH, E_G] bf16
        aggT = {}                   # l -> [EH, NODES]
        diagT = {}                  # l -> [EH, NODES]
        x_cur = {0: x_T}            # l -> x_T tiles
        RPQ = MMQ // N              # receivers per z1-build matmul block (4)

        def emit_ab_proj(l, g):
            gb = slice(g * N, (g + 1) * N)
            xT = x_cur[l]
            apbp = psum.tile([EH, 256], f32, tag="SP", name=f"apbp_{l}_{g}")
            for m in range(HC):
                nc.tensor.matmul(apbp[:, 0:128], lhsT=we1_sb[l][m][:],
                                 rhs=xT[m][:, gb],
                                 start=(m == 0), stop=(m == HC - 1))
            for m in range(HC):
                nc.tensor.matmul(apbp[:, 128:256], lhsT=we1_sb[l][HC + m][:],
                                 rhs=xT[m][:, gb],
                                 start=(m == 0), stop=(m == HC - 1))
            a = work.tile([EH, N], mdt, tag=f"abT{g}", name=f"abT_{l}_{g}")
            nc.vector.tensor_copy(a[:], apbp[:, 0:128])
            abT[(l, g)] = a
            b = work.tile([EH, MMQ], mdt, tag=f"b512_{g}", name=f"b512_{l}_{g}")
            nc.vector.tensor_copy(b[:, 0:128], apbp[:, 128:256])
            nc.vector.tensor_copy(b[:, 128:256], b[:, 0:128])
            nc.vector.tensor_copy(b[:, 256:512], b[:, 0:256])
            b512[(l, g)] = b

        def emit_chunk(l, g, f):
            # build z1 chunk in Z (PE, <=1536 cols), silu1 -> m1buf (ACT)
            r0, nr = CH[f]
            w = nr * N
            c0 = r0 * N
            Z = psum.tile([EH, w], f32, tag="Z", name=f"z_{l}_{g}_{f}")
            for q in range(w // MMQ):
                base = r0 + RPQ * q
                rhs_a = abT[(l, g)][:, base: base + RPQ] \
                    .unsqueeze(2).broadcast_to([EH, RPQ, N])
                nc.tensor.matmul(Z[:, q * MMQ:(q + 1) * MMQ],
                                 lhsT=ident_sb[:], rhs=rhs_a,
                                 start=True, stop=False)
                nc.tensor.matmul(Z[:, q * MMQ:(q + 1) * MMQ],
                                 lhsT=ident_sb[:], rhs=b512[(l, g)][:],
                                 start=False, stop=True)
            nc.scalar.activation(m1buf[g][:, c0:c0 + w], Z[:],
                                 AF.Silu, bias=be1_sb[:, l:l + 1])

        def emit_slab(l, g, s):
            # edge-MLP2 (PE) -> M, silu2 (ACT, FD=S1), reduce+diag (DVE)
            r0 = s * (S1 // N)
            nr = S1 // N
            c0 = s * S1
            M = psum.tile([EH, S1], f32, tag="M", name=f"m_{l}_{g}_{s}")
            for q in range(S1 // MMQ):
                nc.tensor.matmul(M[:, q * MMQ:(q + 1) * MMQ],
                                 lhsT=we2_sb[l][:],
                                 rhs=m1buf[g][:, c0 + q * MMQ: c0 + (q + 1) * MMQ],
                                 start=True, stop=True)
            m2 = mpool.tile([EH, S1], f32, tag="m2", bufs=3, name=f"m2_{l}_{g}_{s}")
            nc.scalar.activation(m2[:], M[:], AF.Silu, bias=be2_sb[:, l:l + 1])
            red_in = m2[:].rearrange("p (i j) -> p i j", j=N)
            nc.vector.tensor_reduce(
                aggT[l][:, g * N + r0: g * N + r0 + nr],
                red_in, axis=mybir.AxisListType.X, op=ALU.add)
            # diagonal (j == i) messages for correction: receiver r0+t's own
            # column within this slab is t*N + (r0+t)
            diag_ap = bass.AP(
                tensor=m2.tensor, offset=m2.offset + r0,
                ap=[m2.ap[0], [N + 1, nr]])
            nc.vector.tensor_copy(
                diagT[l][:, g * N + r0: g * N + r0 + nr], diag_ap)

        def emit_node_mlp(l, g):
            # subtract diagonal, node MLP (bn1 folded via ones-matmul),
            # residual into x_cur[l+1]
            gb = slice(g * N, (g + 1) * N)
            xT = x_cur[l]
            xN = x_cur[l + 1]
            nc.vector.tensor_sub(aggT[l][:, gb], aggT[l][:, gb], diagT[l][:, gb])
            up = psum.tile([128, 256], f32, tag="SP", name=f"up_{l}_{g}")
            for m in range(HC):
                mc = slice(m * 128, (m + 1) * 128)
                nc.tensor.matmul(up[:, mc], lhsT=wn1_sb[l][0][:, m * 128:(m + 1) * 128],
                                 rhs=xT[0][:, gb], start=True, stop=False)
                nc.tensor.matmul(up[:, mc], lhsT=wn1_sb[l][1][:, m * 128:(m + 1) * 128],
                                 rhs=xT[1][:, gb], start=False, stop=False)
                nc.tensor.matmul(up[:, mc], lhsT=wn1_sb[l][2][:, m * 128:(m + 1) * 128],
                                 rhs=aggT[l][:, gb], start=False, stop=False)
                nc.tensor.matmul(up[:, mc],
                                 lhsT=bn1_row[0:1, l * H + m * 128: l * H + (m + 1) * 128],
                                 rhs=ones_sb[0:1, 0:128], start=False, stop=True)
            u1c = work.tile([128, 256], f32, tag="u1c", name=f"u1c_{l}_{g}")
            nc.scalar.activation(u1c[:], up[:], AF.Silu)
            u2 = psum.tile([128, 256], f32, tag="SP", name=f"u2_{l}_{g}")
            for m in range(HC):
                mc = slice(m * 128, (m + 1) * 128)
                nc.tensor.matmul(u2[:, mc], lhsT=wn2_sb[l][0][:, m * 128:(m + 1) * 128],
                                 rhs=u1c[:, 0:128], start=True, stop=False)
                nc.tensor.matmul(u2[:, mc], lhsT=wn2_sb[l][1][:, m * 128:(m + 1) * 128],
                                 rhs=u1c[:, 128:256], start=False, stop=True)
            for m in range(HC):
                mc = slice(m * 128, (m + 1) * 128)
                nc.vector.scalar_tensor_tensor(
                    xN[m][:, gb], u2[:, mc],
                    bn2_sb[:, l * HC + m: l * HC + m + 1], xT[m][:, gb],
                    op0=ALU.add, op1=ALU.add)

        # ---- flat software-pipelined emission across (layer, graph) ----
        for g in range(GPC):
            m1buf[g] = work.tile([EH, E_G], mdt, tag=f"m1_{g}", bufs=1,
                                 name=f"m1buf_{g}")
        for l in range(L):
            aggT[l] = work.tile([EH, NODES], f32, tag="agg", name=f"agg_{l}")
            diagT[l] = work.tile([EH, NODES], f32, tag="diag", name=f"diag_{l}")
            x_cur[l + 1] = [xpool.tile([128, NODES], f32, tag=f"x{m}",
                                       name=f"x{l + 1}_{m}")
                            for m in range(HC)]

        slabs = [(l, g, s) for l in range(L) for g in range(GPC)
                 for s in range(NS1)]
        chunks = [(l, g, f) for l in range(L) for g in range(GPC)
                  for f in range(NCH)]
        covered = {}
        si = ci = 0
        last_was_slab = False
        while si < len(slabs):
            ls, gs, ss = slabs[si]
            emit_c = False
            if ci < len(chunks):
                lc, gc, fc = chunks[ci]
                need = (ss + 1) * S1
                if (lc, gc) == (ls, gs):
                    emit_c = covered.get((ls, gs), 0) < min(need + S2, E_G + 1)
                else:
                    emit_c = last_was_slab
            if emit_c:
                if fc == 0:
                    emit_ab_proj(lc, gc)
                emit_chunk(lc, gc, fc)
                covered[(lc, gc)] = (CH[fc][0] + CH[fc][1]) * N
                ci += 1
                last_was_slab = False
            else:
                emit_slab(ls, gs, ss)
                si += 1
                last_was_slab = True
                if ss == NS1 - 1:
                    emit_node_mlp(ls, gs)
        while ci < len(chunks):   # safety (should not trigger)
            lc, gc, fc = chunks[ci]
            if fc == 0:
                emit_ab_proj(lc, gc)
            emit_chunk(lc, gc, fc)
            ci += 1

        # ---- output embedding: out = x @ w_out + b_out (natural layout) ----
        xF = x_cur[L]
        for nb in range(NODES // 128):
            op_ = psum.tile([128, OUT_NF], f32, tag=("Z", "M")[nb],
                            name=f"outp_{nb}")
            nc.tensor.matmul(op_[:], lhsT=xF[0][:, nb * 128:(nb + 1) * 128],
                             rhs=w_out_sb[0][:], start=True, stop=False)
            nc.tensor.matmul(op_[:], lhsT=xF[1][:, nb * 128:(nb + 1) * 128],
                             rhs=w_out_sb[1][:], start=False, stop=False)
            nc.tensor.matmul(op_[:], lhsT=ones_sb[0:1, 0:128], rhs=b_out_sb[0:1, :],
                             start=False, stop=True)
            ob = work.tile([128, OUT_NF], f32, tag="ob", name=f"ob_{nb}")
            nc.vector.tensor_copy(ob[:], op_[:])
            dma(out=out_d[nb * 128:(nb + 1) * 128, :], in_=ob[:])

    if split_waits:
        _split_excess_waits(nc, mybir)
    return nc


def _get_nc():
    if "nc" not in _CACHE:
        _CACHE["nc"] = _build_nc()
    return _CACHE["nc"]


def _to_mdt(a):
    if MM_DT == "bf16":
        import ml_dtypes
        return np.asarray(a, dtype=np.float32).astype(ml_dtypes.bfloat16)
    return np.asarray(a, dtype=np.float32)


def _run_on_hw(inputs, **spmd_kwargs):
    """Shard, run on the 8 NeuronCores, gather. Returns (out, BassKernelResults)."""
    from concourse.bass_utils import run_bass_kernel_spmd

    f = np.float32
    h = np.ascontiguousarray(np.asarray(inputs["h"], dtype=f))
    ws = {k: np.ascontiguousarray(np.asarray(inputs[k], dtype=f))
          for k in ("w_in", "b_in", "w_out", "b_out", "we1", "be1", "we2",
                    "be2", "wn1", "bn1", "wn2", "bn2")}
    nc = _get_nc()
    base = {
        "w_in": ws["w_in"], "b_in": ws["b_in"],
        "w_out": ws["w_out"], "b_out": ws["b_out"],
        "we1": ws["we1"], "be1": ws["be1"],
        "we2": _to_mdt(ws["we2"]), "be2": ws["be2"],
        "wn1": ws["wn1"], "bn1": ws["bn1"],
        "wn2": ws["wn2"], "bn2": ws["bn2"],
        "ident": _to_mdt(np.eye(128, dtype=f)),
        "identf": np.eye(128, dtype=f),
        "ones_r": np.ones((1, 128), dtype=f),
    }
    in_maps = []
    for c in range(NCORES):
        m = dict(base)
        m["h_c"] = np.ascontiguousarray(
            h[c * GPC:(c + 1) * GPC].reshape(NODES, IN_NF))
        in_maps.append(m)

    res = run_bass_kernel_spmd(nc, in_maps, list(range(NCORES)), **spmd_kwargs)
    out = np.concatenate([np.asarray(res.results[i]["out_c"], dtype=f)
                          for i in range(NCORES)], axis=0)
    return out, res


def kernel(**inputs):
    h = np.asarray(inputs["h"])
    rows, cols = inputs["rows"], inputs["cols"]
    if h.shape != (BS, N, IN_NF) or not _edges_match(rows, cols):
        ws = {k: np.asarray(inputs[k], dtype=np.float32)
              for k in ("w_in", "b_in", "w_out", "b_out", "we1", "be1", "we2",
                        "be2", "wn1", "bn1", "wn2", "bn2")}
        return _numpy_reference(np.asarray(h, np.float32), rows, cols, **ws)
    out, _ = _run_on_hw(inputs)
    return out
